# revision 1
# baseline (speedup 1.0000x reference)
"""Trainium2 Bass kernel for CGL contrastive region loss.

Problem: proj (96, 256, 64, 64) f32 = 3 stacked views of B=32 images.
Only views 2 and 3 (aug1/aug2) are used. From each image, 25 regions
(5x5 grid of 2x2 windows at centres {10..50}) are extracted over all 256
channels -> region vectors of D = 256*2*2 = 1024. Per image pair the loss
needs the 50x50 Gram matrix of the stacked normalized region vectors
[u1; u2]: S11/S22/S12/S21, positive logits, and two masked logsumexps.
The scalar loss is a sum over pairs -> data-parallel over batch.

Mapping (4 pairs/core on 8 cores):
  host: gather 2x2 windows into a (128, pairs*8*50) array per core:
        partition = channel%128, free = (pair, chunk k=(cb,dy,dx), view, region)
  device, per pair b:
        G_b (50x50, PSUM) = sum_k U_bk^T U_bk   (8 matmuls, K=128, f32r 1-pass)
  batched across pairs (blocks side by side in a (50, 200) layout):
        d    = diag(G)                       (squared region norms)
        inv  = sqrt(10)*rsqrt(d)             (Newton rsqrt on DVE; folds 1/TAU)
        S    = G * inv_row * inv_col         (col-scale broadcast via one PE matmul)
        msub = S + M  where M = -1e30 on diag else -10
          (S diag == 1/TAU exactly, so shifting by the constant -10 is a safe
           logsumexp max-shift; no row-max reduce needed. The +10 constants in
           lse and -2*10 in pos cancel in sum(lse-pos).)
        pos' = diag(msub[0:25, 25:50] blocks)
        lnes = ln(sum(exp(msub)))            (exp + ln on ACT)
        total = sum(lnes) - 2*sum(pos')
  output: per-core scalar total / (2*R*B_global); host sums 8 partials.

ACT runs Exp then Ln; the Exp table load overlaps the input DMAs and the
Ln table load overlaps the DVE esum reduce. Inputs are split into four
quarter-DMAs across the sync/scalar HWDGE rings (per-pair tiles so each
gram chain starts as its quarter lands); consts ride the gpsimd SWDGE
ring. Gram + broadcast matmuls use float32r (~13-bit mantissa). rsqrt:
quake init + 2 Newton steps on DVE, hidden norm extraction under the gram
phase. The per-core scalar partial is reduced over partitions by a final
PE matmul; the host sums the 8 cores.
"""

import numpy as np

NB = 4                    # pairs per core
NCORES = 8
R = 25
FREE = NB * 8 * 50        # 1600 free elements per core
_CENTRES = (10, 20, 30, 40, 50)

# degree-6 power-basis fit of log2(m), m in [1,2), max abs err 5.1e-6
# (a0 has the -127 exponent-bias correction folded in)
_LOG2_COEF = (
    -3.0283249744104577 - 127.0, 6.065858861121359, -5.264155524116715,
    3.218869813800031, -1.234279899429953, 0.26686276780638246,
    -0.024825984442692788,
)
_LN2 = 0.6931471805599453
_SQRT10 = 3.1622776601683795

# const tensor layout: (50, 406) f32
#   [0:200) ident4: 4 horizontally tiled 50x50 identities
#   [200:400) maskM4: -1e30 on each block diagonal, -10.0 elsewhere
#   [400:406) int32 bit patterns (f32-bitcast): 1, 23, 127, 0x5f3759df,
#             0x007fffff, 0x3f800000
#   [406] 1.0f (ones column for the final partition-sum matmul)
_CF_COLS = 407
_ICONST = (1, 23, 127, 0x5F3759DF, 0x007FFFFF, 0x3F800000)

_nc_cache = None


def _build_consts():
    ident = np.eye(50, dtype=np.float32)
    cf = np.zeros((50, _CF_COLS), dtype=np.float32)
    for b in range(4):
        cf[:, b * 50 : (b + 1) * 50] = ident
        cf[:, 200 + b * 50 : 200 + (b + 1) * 50] = np.where(
            ident > 0, np.float32(-1e30), np.float32(-10.0)
        )
    icol = np.array(_ICONST, dtype=np.int32).view(np.float32)
    cf[:, 400:406] = icol[None, :]
    cf[:, 406] = 1.0
    cr = np.ones((50, 51), dtype=np.float32)
    return cf, cr


def _build_nc():
    import concourse.bacc as bacc
    import concourse.tile as tile
    from concourse import mybir
    from concourse.vector_clock import ScopedClock

    class FastTailTileContext(tile.TileContext):
        """Tile tail without the two full all-engine barriers.

        The sync-engine drain already waits on the global vector clock
        (every instruction's sem tick), so once it completes nothing is
        in flight; a sem-only EVSEM barrier then orders the gpsimd
        sem_clears after it. Saves ~6us of kernel tail."""

        def _drain_and_barrier(self, tick_clock, wait_clock):
            drain_inst = self.nc.sync.drain()
            wait_clock.add_sem_waits(
                drain_inst.ins, ScopedClock({None: tick_clock.global_clock})
            )
            self.nc.all_engine_barrier(sem_only=True)
            popped = self.nc._tile_sem_poison_stack.pop()
            assert popped is self._sem_poison
            self.nc.clear_and_free_semaphores(list(self.sems.allocated().values()))

    f32 = mybir.dt.float32
    f32r = mybir.dt.float32r
    i32 = mybir.dt.int32
    Alu = mybir.AluOpType
    Act = mybir.ActivationFunctionType
    X = mybir.AxisListType.X

    nc = bacc.Bacc("TRN2", target_bir_lowering=False, debug=False)
    u_dram = nc.dram_tensor("u", [128, FREE], f32r, kind="ExternalInput").ap()
    cf_dram = nc.dram_tensor("cf", [50, _CF_COLS], f32, kind="ExternalInput").ap()
    cr_dram = nc.dram_tensor("cr", [50, 51], f32r, kind="ExternalInput").ap()
    out_dram = nc.dram_tensor("out", [1, 1], f32, kind="ExternalOutput").ap()

    def blk(ap, f=50):
        return ap.rearrange("p (b f) -> p b f", f=f)

    with FastTailTileContext(nc) as tc:
        with (
            tc.tile_pool(name="data", bufs=1) as data,
            tc.tile_pool(name="consts", bufs=1) as consts,
            tc.tile_pool(name="work", bufs=2) as work,
            tc.tile_pool(name="psg", bufs=4, space="PSUM") as psg,
            tc.tile_pool(name="psb", bufs=1, space="PSUM") as psb,
            tc.tile_pool(name="pst", bufs=1, space="PSUM") as pst,
        ):
            Q = FREE // 4
            ubs = []
            for b in range(NB):
                ubq = data.tile([128, Q], f32r, tag=f"ub{b}")
                eng = nc.sync if b % 2 == 0 else nc.scalar
                eng.dma_start(ubq[:], u_dram[:, b * Q : (b + 1) * Q])
                ubs.append(ubq)
            cf = consts.tile([50, _CF_COLS], f32)
            nc.gpsimd.dma_start(cf[:], cf_dram)
            cr = consts.tile([50, 51], f32r)
            nc.gpsimd.dma_start(cr[:], cr_dram)

            ident4 = cf[:, 0:200]
            maskM4 = cf[:, 200:400]
            cfi = cf[:].bitcast(i32)
            c_sh1, c_sh23, c_127, c_magic, c_mant, c_one = (
                cfi[:, 400 + j : 401 + j] for j in range(6)
            )
            ones50r = cr[:, 0:50]
            ones_col_f = cf[:, 406:407]

            dmul = work.tile([50, 200], f32, tag="dmul")
            gps = []
            for b in range(NB):
                gp = psg.tile([50, 50], f32, tag="g")
                for k in range(8):
                    sl = ubs[b][:, k * 50 : (k + 1) * 50]
                    nc.tensor.matmul(gp[:], sl, sl, start=(k == 0), stop=(k == 7))
                gps.append(gp)

            # squared norms from block diagonals (hidden under the gram phase)
            dsq = work.tile([50, NB], f32, tag="dsq")
            for b in range(NB):
                nc.vector.tensor_mul(
                    dmul[:, b * 50 : (b + 1) * 50], gps[b][:],
                    ident4[:, b * 50 : (b + 1) * 50],
                )
                nc.vector.reduce_sum(
                    dsq[:, b : b + 1],
                    dmul[:, b * 50 : (b + 1) * 50].unsqueeze(1),
                    axis=X,
                )

            # inv = sqrt(10) * rsqrt(d): quake initial guess + 2 Newton steps
            t2 = work.tile([50, NB], i32, tag="t2")
            nc.vector.tensor_tensor(
                t2[:], dsq[:].bitcast(i32), c_sh1.broadcast_to([50, NB]),
                op=Alu.logical_shift_right,
            )  # xi >> 1
            nc.vector.tensor_sub(
                t2[:], c_magic.broadcast_to([50, NB]), t2[:]
            )  # magic - (xi >> 1)
            y0 = t2[:].bitcast(f32)
            ya = work.tile([50, NB], f32, tag="ya")
            nc.vector.tensor_mul(ya[:], y0, y0)
            nc.vector.tensor_mul(ya[:], ya[:], dsq[:])
            yc = work.tile([50, NB], f32, tag="yc")
            nc.vector.tensor_scalar(yc[:], ya[:], -0.5, 1.5, op0=Alu.mult, op1=Alu.add)
            y1 = work.tile([50, NB], f32, tag="y1")
            nc.vector.tensor_mul(y1[:], y0, yc[:])
            nc.vector.tensor_mul(ya[:], y1[:], y1[:])
            nc.vector.tensor_mul(ya[:], ya[:], dsq[:])
            nc.vector.tensor_scalar(yc[:], ya[:], -0.5, 1.5, op0=Alu.mult, op1=Alu.add)
            inv = work.tile([50, NB], f32, tag="inv")
            nc.vector.scalar_tensor_tensor(
                inv[:], y1[:], _SQRT10, yc[:], op0=Alu.mult, op1=Alu.mult
            )

            # scale: S = G * inv_row * inv_col; col-broadcast via ones^T @ diag(inv)
            invrep = inv[:].unsqueeze(2).broadcast_to([50, NB, 50])
            dinv = work.tile([50, 200], f32r, tag="dinv")
            nc.vector.tensor_mul(blk(dinv[:]), blk(ident4), invrep)
            binv4 = psb.tile([50, 200], f32, tag="binv4")
            nc.tensor.matmul(binv4[:], ones50r, dinv[:], start=True, stop=True)
            rowsc = work.tile([50, 200], f32, tag="rowsc")
            for b in range(NB):
                nc.vector.tensor_mul(
                    rowsc[:, b * 50 : (b + 1) * 50].unsqueeze(1),
                    gps[b][:].unsqueeze(1),
                    invrep[:, b : b + 1, :],
                )
            msub = work.tile([50, 200], f32, tag="msub")
            nc.vector.tensor_mul(msub[:], rowsc[:], binv4[:])
            nc.vector.tensor_add(msub[:], msub[:], maskM4)

            # positives (shifted by -10; constant cancels in the final sum)
            pmul = work.tile([25, NB * 25], f32, tag="pmul")
            nc.vector.tensor_mul(
                blk(pmul[:], f=25),
                blk(msub[0:25, :])[:, :, 25:50],
                blk(ident4[0:25])[:, :, 0:25],
            )
            posf = consts.tile([50, NB], f32)
            nc.vector.memset(posf[:], 0.0)
            nc.vector.reduce_sum(posf[0:25, :], blk(pmul[:], f=25), axis=X)

            # exp on ACT (single function -> single table load, off-path)
            eall = work.tile([50, 200], f32, tag="eall")
            nc.scalar.activation(eall[:], msub[:], Act.Exp)
            esum = work.tile([50, NB], f32, tag="esum")
            nc.vector.reduce_sum(esum[:], blk(eall[:]), axis=X)

            # ln(esum) on ACT: the Ln table load is emitted before the
            # wait-on-esum, so it overlaps the DVE reduce above
            lnes = work.tile([50, NB], f32, tag="lnes")
            nc.scalar.activation(lnes[:], esum[:], Act.Ln)
            # total = sum(lnes) - 2*sum(pos')
            lsesum = work.tile([50, 1], f32, tag="lsesum")
            nc.vector.reduce_sum(lsesum[:], lnes[:], axis=X)
            possum = work.tile([50, 1], f32, tag="possum")
            nc.vector.reduce_sum(possum[:], posf[:], axis=X)
            acc = work.tile([50, 1], f32, tag="acc")
            nc.vector.scalar_tensor_tensor(
                acc[:], possum[:], -2.0, lsesum[:], op0=Alu.mult, op1=Alu.add
            )
            tp = pst.tile([1, 1], f32, tag="tot")
            nc.tensor.matmul(tp[:], acc[:], ones_col_f, start=True, stop=True)
            res = work.tile([1, 1], f32, tag="res")
            nc.vector.tensor_scalar_mul(
                res[:], tp[:], 1.0 / (2.0 * R * NB * NCORES)
            )
            nc.sync.dma_start(out_dram, res[:])

    nc.compile()
    return nc


def get_nc():
    global _nc_cache
    if _nc_cache is None:
        _nc_cache = _build_nc()
    return _nc_cache


def pack_inputs(proj: np.ndarray) -> np.ndarray:
    """(96,256,64,64) -> (128, 32, 8, 50): partition=c%128,
    free=(pair, chunk k=(cb,dy,dx), view, region rh*5+rw)."""
    win = np.array([[c - 1, c] for c in _CENTRES])  # (5, 2): rows/cols of each window
    v = np.stack([proj[32:64], proj[64:96]], axis=1)  # (32, 2, 256, 64, 64)
    g = v[:, :, :, win[:, :, None, None], win[None, None, :, :]]  # (32,2,256,5,2,5,2)
    g = g.reshape(32, 2, 2, 128, 5, 2, 5, 2)  # b, view, cb, c', rh, dy, rw, dx
    arr = np.transpose(g, (3, 0, 2, 5, 7, 1, 4, 6))  # c', b, cb, dy, dx, view, rh, rw
    return np.ascontiguousarray(arr).reshape(128, 32, 8, 50)


def kernel(proj: np.ndarray) -> np.ndarray:
    from concourse.bass_utils import run_bass_kernel_spmd

    nc = get_nc()
    arr = pack_inputs(np.asarray(proj))
    cf, cr = _build_consts()
    in_maps = [
        {
            "u": np.ascontiguousarray(arr[:, c * NB : (c + 1) * NB]).reshape(128, FREE),
            "cf": cf,
            "cr": cr,
        }
        for c in range(NCORES)
    ]
    results = run_bass_kernel_spmd(nc, in_maps, list(range(NCORES))).results
    total = 0.0
    for r in results:
        total += float(r["out"][0, 0])
    return np.float32(total)



# revision 2
# speedup vs baseline: 1.0324x; 1.0324x over previous
"""Trainium2 Bass kernel for CGL contrastive region loss — v3.

Problem: proj (96, 256, 64, 64) f32 = 3 stacked views of B=32 images.
Views 2/3 are used; 25 regions (5x5 grid of 2x2 windows, all 256 chans)
per image -> region vectors D=1024. Per pair the loss needs the 50x50
Gram of [u1;u2] normalized rows, two masked logsumexps, positives.
Scalar loss sums over pairs -> data-parallel over batch, 4 pairs/core.

vs the 35us v1:
  - inputs packed bf16 on host: halves DMA bytes; bf16 gram matmuls
    (1-pass, ~2.5x the fp32 HIGH rate).
  - const traffic is one 10KB (50,50) identity: the 4-block ident and
    mask views are stride-0 broadcast APs, the mask is derived on DVE,
    and the ones matrix is a memset — the input quarters get the SDMA
    engines essentially to themselves.
  - all 4 pair grams accumulate into one (50,200) PSUM tile; the diag
    extraction / row-scale / mask / exp run as single 200-wide ops.
  - rsqrt via ACT: inv = exp(-0.5*ln(0.1*d)) — two tiny activations
    replacing the 12-op DVE Newton chain. The mask (-1e30 diag, -10
    shift) stays: ACT-table inv error (~1e-3) perturbs the diagonal
    logit by ~0.02, so the diag must be hard-masked.
  - insert_act_table_loads is steered to the one act-func set that
    contains BOTH exp and ln ("natural_log_exp_and_others"), so the
    kernel does exactly one table load, hoisted behind a no-wait dummy
    exp into the DMA window (v1/v2 paid 1.3-2.6us of mid-chain loads).
  - per-core output is the (50,8) [lnes | posf] tile; final scalar
    assembly happens on the host during the gather step.
"""

import numpy as np
import ml_dtypes

NB = 4                    # pairs per core
NCORES = 8
R = 25
FREE = NB * 8 * 50        # 1600 free elements per core
Q = FREE // NB            # 400 per pair
_CENTRES = (10, 20, 30, 40, 50)
_BOTH_SET = "natural_log_exp_and_others"

_nc_cache = None


def _build_nc():
    import concourse.bacc as bacc
    import concourse.tile as tile
    from concourse import mybir
    from concourse.hw_specs import get_activation_tables
    from concourse.vector_clock import ScopedClock

    class FastTailTileContext(tile.TileContext):
        """Tile tail without the two full all-engine barriers.

        The sync-engine drain already waits on the global vector clock
        (every instruction's sem tick), so once it completes nothing is
        in flight; a sem-only EVSEM barrier then orders the gpsimd
        sem_clears after it. Saves ~6us of kernel tail."""

        def _drain_and_barrier(self, tick_clock, wait_clock):
            drain_inst = self.nc.sync.drain()
            wait_clock.add_sem_waits(
                drain_inst.ins, ScopedClock({None: tick_clock.global_clock})
            )
            self.nc.all_engine_barrier(sem_only=True)
            popped = self.nc._tile_sem_poison_stack.pop()
            assert popped is self._sem_poison
            self.nc.clear_and_free_semaphores(list(self.sems.allocated().values()))

    class OneActSetBacc(bacc.Bacc):
        """Steer activation-table selection to the single set holding
        both exp and ln, so the kernel needs exactly one table load.

        The act_func_set_id written on InstLoadActFuncSet is the INDEX
        into act_info.json's act_func_sets, so the list order must be
        preserved — other sets are emptied, not removed, which makes
        them unselectable without disturbing the indices."""

        def insert_act_table_loads(self):
            has_activation = any(
                isinstance(i, mybir.InstActivation)
                for b in self.main_func.blocks
                for i in b.instructions
            )
            if not has_activation:
                return
            tables = [
                (name, funcs if name == _BOTH_SET else set())
                for name, funcs in get_activation_tables(self.m.arch).items()
            ]
            bacc._bass_rust.insert_act_table_loads(self, tables)

    f32 = mybir.dt.float32
    bf16 = mybir.dt.bfloat16
    Act = mybir.ActivationFunctionType
    Alu = mybir.AluOpType
    X = mybir.AxisListType.X

    nc = OneActSetBacc("TRN2", target_bir_lowering=False, debug=False)
    u_dram = nc.dram_tensor("u", [128, FREE], bf16, kind="ExternalInput").ap()
    cf_dram = nc.dram_tensor("cf", [50, 50], f32, kind="ExternalInput").ap()
    out_dram = nc.dram_tensor("out", [50, 8], f32, kind="ExternalOutput").ap()

    def blk(ap, f=50):
        return ap.rearrange("p (b f) -> p b f", f=f)

    with FastTailTileContext(nc) as tc:
        with (
            tc.tile_pool(name="data", bufs=1) as data,
            tc.tile_pool(name="consts", bufs=1) as consts,
            tc.tile_pool(name="work", bufs=2) as work,
            tc.tile_pool(name="psg", bufs=1, space="PSUM") as psg,
            tc.tile_pool(name="psb", bufs=1, space="PSUM") as psb,
        ):
            # input DMAs: one quarter per pair, two per HWDGE ring
            ubs = [
                data.tile([128, Q], bf16, name=f"ub{b}", tag=f"ub{b}")
                for b in range(NB)
            ]
            for b, eng in ((0, nc.sync), (1, nc.scalar), (2, nc.sync), (3, nc.scalar)):
                eng.dma_start(ubs[b][:], u_dram[:, b * Q : (b + 1) * Q])

            # dummy exp on a constant tile: hoists the single ACT table
            # load into the DMA window instead of the post-gram chain
            dummy = consts.tile([1, 1], f32)
            nc.vector.memset(dummy[:], 1.0)
            dume = work.tile([1, 1], f32, tag="dume")
            nc.scalar.activation(dume[:], dummy[:], Act.Exp)

            # consts: (50,50) identity from DRAM; mask derived on DVE
            # (idle during the input DMAs); 4-block views are stride-0
            # broadcast APs
            ident = consts.tile([50, 50], f32)
            nc.gpsimd.dma_start(ident[:], cf_dram)
            maskM = consts.tile([50, 50], f32)
            nc.vector.tensor_scalar(
                maskM[:], ident[:], -1e30, -10.0, op0=Alu.mult, op1=Alu.add
            )
            identB = ident[:].unsqueeze(1).broadcast_to([50, NB, 50])
            maskB = maskM[:].unsqueeze(1).broadcast_to([50, NB, 50])
            cb = consts.tile([50, 50], bf16)
            nc.vector.memset(cb[:], 1.0)

            # output tile; posf lands in cols 4:8 (rows 25:50 stay 0)
            cmb = consts.tile([50, 8], f32)
            nc.vector.memset(cmb[:, 4:8], 0.0)

            # grams: 4 accumulation groups into one (50,200) PSUM tile,
            # each followed by a mul+reduce pair that pulls the block
            # diagonal (squared norms) while the next pair still matmuls
            gp = psg.tile([50, 200], f32, tag="g")
            dmul = work.tile([50, 200], f32, tag="dmul")
            dsq = work.tile([50, NB], f32, tag="dsq")
            for b in range(NB):
                for k in range(8):
                    sl = ubs[b][:, k * 50 : (k + 1) * 50]
                    nc.tensor.matmul(
                        gp[:, b * 50 : (b + 1) * 50], sl, sl,
                        start=(k == 0), stop=(k == 7),
                    )
                nc.vector.tensor_mul(
                    dmul[:, b * 50 : (b + 1) * 50],
                    gp[:, b * 50 : (b + 1) * 50],
                    ident[:],
                )
                nc.vector.reduce_sum(
                    dsq[:, b : b + 1],
                    dmul[:, b * 50 : (b + 1) * 50].unsqueeze(1),
                    axis=X,
                )

            # inv = sqrt(10)*rsqrt(d) = exp(-0.5*ln(0.1*d)) on ACT
            tln = work.tile([50, NB], f32, tag="tln")
            nc.scalar.activation(tln[:], dsq[:], Act.Ln, scale=0.1)
            inv = work.tile([50, NB], f32, tag="inv")
            nc.scalar.activation(inv[:], tln[:], Act.Exp, scale=-0.5)

            # S = G * inv_row * inv_col; col-broadcast via ones^T @ diag(inv).
            # dinv first: the PE matmul it feeds overlaps the DVE row-scale.
            invrep = inv[:].unsqueeze(2).broadcast_to([50, NB, 50])
            dinv = work.tile([50, 200], bf16, tag="dinv")
            nc.vector.tensor_mul(blk(dinv[:]), identB, invrep)
            binv4 = psb.tile([50, 200], f32, tag="binv4")
            nc.tensor.matmul(binv4[:], cb[:], dinv[:], start=True, stop=True)
            rowsc = work.tile([50, 200], f32, tag="rowsc")
            nc.vector.tensor_mul(blk(rowsc[:]), blk(gp[:]), invrep)
            msub = work.tile([50, 200], f32, tag="msub")
            nc.vector.tensor_mul(msub[:], rowsc[:], binv4[:])
            nc.vector.tensor_add(blk(msub[:]), blk(msub[:]), maskB)

            eall = work.tile([50, 200], f32, tag="eall")
            nc.scalar.activation(eall[:], msub[:], Act.Exp)
            esum = work.tile([50, NB], f32, tag="esum")
            nc.vector.reduce_sum(esum[:], blk(eall[:]), axis=X)
            nc.scalar.activation(cmb[:, 0:4], esum[:], Act.Ln)

            # positives (diag of each pair's (0:25, 25:50) block, -10
            # shifted by the mask): emitted after esum so the DVE runs
            # them during the final ln, off the critical path
            pmul = work.tile([25, NB * 25], f32, tag="pmul")
            nc.vector.tensor_mul(
                blk(pmul[:], f=25),
                blk(msub[0:25, :])[:, :, 25:50],
                ident[0:25, 0:25].unsqueeze(1).broadcast_to([25, NB, 25]),
            )
            nc.vector.reduce_sum(cmb[0:25, 4:8], blk(pmul[:], f=25), axis=X)

            nc.sync.dma_start(out_dram, cmb[:])

    nc.compile()
    return nc


def get_nc():
    global _nc_cache
    if _nc_cache is None:
        _nc_cache = _build_nc()
    return _nc_cache


def pack_inputs(proj: np.ndarray) -> np.ndarray:
    """(96,256,64,64) -> (128, 32, 8, 50) bf16: partition=c%128,
    free=(pair, chunk k=(cb,dy,dx), view, region rh*5+rw)."""
    win = np.array([[c - 1, c] for c in _CENTRES])  # (5, 2): rows/cols of window
    v = np.stack([proj[32:64], proj[64:96]], axis=1)  # (32, 2, 256, 64, 64)
    g = v[:, :, :, win[:, :, None, None], win[None, None, :, :]]  # (32,2,256,5,2,5,2)
    g = g.reshape(32, 2, 2, 128, 5, 2, 5, 2)  # b, view, cb, c', rh, dy, rw, dx
    arr = np.transpose(g, (3, 0, 2, 5, 7, 1, 4, 6))  # c', b, cb, dy, dx, view, rh, rw
    return np.ascontiguousarray(arr).reshape(128, 32, 8, 50).astype(ml_dtypes.bfloat16)


def kernel(proj: np.ndarray) -> np.ndarray:
    from concourse.bass_utils import run_bass_kernel_spmd

    nc = get_nc()
    arr = pack_inputs(np.asarray(proj))
    cf = np.eye(50, dtype=np.float32)
    in_maps = [
        {
            "u": np.ascontiguousarray(arr[:, c * NB : (c + 1) * NB]).reshape(128, FREE),
            "cf": cf,
        }
        for c in range(NCORES)
    ]
    results = run_bass_kernel_spmd(nc, in_maps, list(range(NCORES))).results
    total = 0.0
    for r in results:
        out = np.asarray(r["out"], dtype=np.float64)  # (50, 8)
        lnes = out[:, 0:4]          # lse - 10 per (row, pair)
        posf = out[0:25, 4:8]       # positive logits - 10 per (region, pair)
        total += lnes.sum() - 2.0 * posf.sum()
    return np.float32(total / (2.0 * R * NB * NCORES))


# revision 3
# speedup vs baseline: 1.0558x; 1.0227x over previous
"""Trainium2 Bass kernel for CGL contrastive region loss — v3.

Problem: proj (96, 256, 64, 64) f32 = 3 stacked views of B=32 images.
Views 2/3 are used; 25 regions (5x5 grid of 2x2 windows, all 256 chans)
per image -> region vectors D=1024. Per pair the loss needs the 50x50
Gram of [u1;u2] normalized rows, two masked logsumexps, positives.
Scalar loss sums over pairs -> data-parallel over batch, 4 pairs/core.

vs the 35us v1:
  - inputs packed bf16 on host: halves DMA bytes; bf16 gram matmuls
    (1-pass, ~2.5x the fp32 HIGH rate).
  - const traffic is one 10KB (50,50) identity: the 4-block ident and
    mask views are stride-0 broadcast APs, the mask is derived on DVE,
    and the ones matrix is a memset — the input quarters get the SDMA
    engines essentially to themselves.
  - all 4 pair grams accumulate into one (50,200) PSUM tile; the diag
    extraction / row-scale / mask / exp run as single 200-wide ops.
  - rsqrt via ACT: inv = exp(-0.5*ln(0.1*d)) — two tiny activations
    replacing the 12-op DVE Newton chain. The mask (-1e30 diag, -10
    shift) stays: ACT-table inv error (~1e-3) perturbs the diagonal
    logit by ~0.02, so the diag must be hard-masked.
  - insert_act_table_loads is steered to the one act-func set that
    contains BOTH exp and ln ("natural_log_exp_and_others"), so the
    kernel does exactly one table load, hoisted behind a no-wait dummy
    exp into the DMA window (v1/v2 paid 1.3-2.6us of mid-chain loads).
  - per-core output is the (50,8) [lnes | posf] tile; final scalar
    assembly happens on the host during the gather step.
"""

import numpy as np
import ml_dtypes

NB = 4                    # pairs per core
NCORES = 8
R = 25
FREE = NB * 8 * 50        # 1600 free elements per core
Q = FREE // NB            # 400 per pair
_CENTRES = (10, 20, 30, 40, 50)
_BOTH_SET = "natural_log_exp_and_others"

_nc_cache = None


def _build_nc():
    import concourse.bacc as bacc
    import concourse.tile as tile
    from concourse import mybir
    from concourse.hw_specs import get_activation_tables
    from concourse.vector_clock import ScopedClock

    class FastTailTileContext(tile.TileContext):
        """Tile tail without the two full all-engine barriers.

        The sync-engine drain already waits on the global vector clock
        (every instruction's sem tick), so once it completes nothing is
        in flight; a sem-only EVSEM barrier then orders the gpsimd
        sem_clears after it. Saves ~6us of kernel tail."""

        def _drain_and_barrier(self, tick_clock, wait_clock):
            drain_inst = self.nc.sync.drain()
            wait_clock.add_sem_waits(
                drain_inst.ins, ScopedClock({None: tick_clock.global_clock})
            )
            self.nc.all_engine_barrier(sem_only=True)
            popped = self.nc._tile_sem_poison_stack.pop()
            assert popped is self._sem_poison
            self.nc.clear_and_free_semaphores(list(self.sems.allocated().values()))

    class OneActSetBacc(bacc.Bacc):
        """Steer activation-table selection to the single set holding
        both exp and ln, so the kernel needs exactly one table load.

        The act_func_set_id written on InstLoadActFuncSet is the INDEX
        into act_info.json's act_func_sets, so the list order must be
        preserved — other sets are emptied, not removed, which makes
        them unselectable without disturbing the indices."""

        def insert_act_table_loads(self):
            has_activation = any(
                isinstance(i, mybir.InstActivation)
                for b in self.main_func.blocks
                for i in b.instructions
            )
            if not has_activation:
                return
            tables = [
                (name, funcs if name == _BOTH_SET else set())
                for name, funcs in get_activation_tables(self.m.arch).items()
            ]
            bacc._bass_rust.insert_act_table_loads(self, tables)

    f32 = mybir.dt.float32
    bf16 = mybir.dt.bfloat16
    Act = mybir.ActivationFunctionType
    Alu = mybir.AluOpType
    X = mybir.AxisListType.X

    nc = OneActSetBacc("TRN2", target_bir_lowering=False, debug=False)
    u_dram = nc.dram_tensor("u", [128, FREE], bf16, kind="ExternalInput").ap()
    cf_dram = nc.dram_tensor("cf", [50, 50], f32, kind="ExternalInput").ap()
    out_dram = nc.dram_tensor("out", [50, 8], f32, kind="ExternalOutput").ap()

    def blk(ap, f=50):
        return ap.rearrange("p (b f) -> p b f", f=f)

    with FastTailTileContext(nc) as tc:
        with (
            tc.tile_pool(name="data", bufs=1) as data,
            tc.tile_pool(name="consts", bufs=1) as consts,
            tc.tile_pool(name="work", bufs=2) as work,
            tc.tile_pool(name="psg", bufs=1, space="PSUM") as psg,
            tc.tile_pool(name="psb", bufs=1, space="PSUM") as psb,
        ):
            # input DMAs: one quarter per pair, two per HWDGE ring
            ubs = [
                data.tile([128, Q], bf16, name=f"ub{b}", tag=f"ub{b}")
                for b in range(NB)
            ]
            for b, eng in ((0, nc.sync), (1, nc.scalar), (2, nc.sync), (3, nc.scalar)):
                eng.dma_start(ubs[b][:], u_dram[:, b * Q : (b + 1) * Q])

            # dummy exp on a constant tile: hoists the single ACT table
            # load into the DMA window instead of the post-gram chain
            dummy = consts.tile([1, 1], f32)
            nc.vector.memset(dummy[:], 1.0)
            dume = work.tile([1, 1], f32, tag="dume")
            nc.scalar.activation(dume[:], dummy[:], Act.Exp)

            # consts: (50,50) identity from DRAM; mask derived on DVE
            # (idle during the input DMAs); 4-block views are stride-0
            # broadcast APs
            ident = consts.tile([50, 50], f32)
            nc.gpsimd.dma_start(ident[:], cf_dram)
            maskM = consts.tile([50, 50], f32)
            nc.vector.tensor_scalar(
                maskM[:], ident[:], -1e30, -10.0, op0=Alu.mult, op1=Alu.add
            )
            identB = ident[:].unsqueeze(1).broadcast_to([50, NB, 50])
            maskB = maskM[:].unsqueeze(1).broadcast_to([50, NB, 50])
            cb = consts.tile([50, 50], bf16)
            nc.vector.memset(cb[:], 1.0)

            # output tile; posf lands in cols 4:8 (rows 25:50 stay 0)
            cmb = consts.tile([50, 8], f32)
            nc.vector.memset(cmb[:, 4:8], 0.0)

            # grams: 4 accumulation groups into one (50,200) PSUM tile
            gp = psg.tile([50, 200], f32, tag="g")
            for b in range(NB):
                for k in range(8):
                    sl = ubs[b][:, k * 50 : (k + 1) * 50]
                    nc.tensor.matmul(
                        gp[:, b * 50 : (b + 1) * 50], sl, sl,
                        start=(k == 0), stop=(k == 7),
                    )

            # squared norms from the block diagonals
            dmul = work.tile([50, 200], f32, tag="dmul")
            nc.vector.tensor_mul(blk(dmul[:]), blk(gp[:]), identB)
            dsq = work.tile([50, NB], f32, tag="dsq")
            nc.vector.reduce_sum(dsq[:], blk(dmul[:]), axis=X)

            # inv = sqrt(10)*rsqrt(d) = exp(-0.5*ln(0.1*d)) on ACT
            tln = work.tile([50, NB], f32, tag="tln")
            nc.scalar.activation(tln[:], dsq[:], Act.Ln, scale=0.1)
            inv = work.tile([50, NB], f32, tag="inv")
            nc.scalar.activation(inv[:], tln[:], Act.Exp, scale=-0.5)

            # S = G * inv_row * inv_col; col-broadcast via ones^T @ diag(inv).
            # dinv first: the PE matmul it feeds overlaps the DVE row-scale.
            invrep = inv[:].unsqueeze(2).broadcast_to([50, NB, 50])
            dinv = work.tile([50, 200], bf16, tag="dinv")
            nc.vector.tensor_mul(blk(dinv[:]), identB, invrep)
            binv4 = psb.tile([50, 200], f32, tag="binv4")
            nc.tensor.matmul(binv4[:], cb[:], dinv[:], start=True, stop=True)
            rowsc = work.tile([50, 200], f32, tag="rowsc")
            nc.vector.tensor_mul(blk(rowsc[:]), blk(gp[:]), invrep)
            msub = work.tile([50, 200], f32, tag="msub")
            nc.vector.tensor_mul(msub[:], rowsc[:], binv4[:])
            nc.vector.tensor_add(blk(msub[:]), blk(msub[:]), maskB)

            eall = work.tile([50, 200], f32, tag="eall")
            nc.scalar.activation(eall[:], msub[:], Act.Exp)
            esum = work.tile([50, NB], f32, tag="esum")
            nc.vector.reduce_sum(esum[:], blk(eall[:]), axis=X)
            nc.scalar.activation(cmb[:, 0:4], esum[:], Act.Ln)

            # positives (diag of each pair's (0:25, 25:50) block, -10
            # shifted by the mask): emitted after esum so the DVE runs
            # them during the final ln, off the critical path
            pmul = work.tile([25, NB * 25], f32, tag="pmul")
            nc.vector.tensor_mul(
                blk(pmul[:], f=25),
                blk(msub[0:25, :])[:, :, 25:50],
                ident[0:25, 0:25].unsqueeze(1).broadcast_to([25, NB, 25]),
            )
            nc.vector.reduce_sum(cmb[0:25, 4:8], blk(pmul[:], f=25), axis=X)

            nc.sync.dma_start(out_dram, cmb[:])

    nc.compile()
    return nc


def get_nc():
    global _nc_cache
    if _nc_cache is None:
        _nc_cache = _build_nc()
    return _nc_cache


def pack_inputs(proj: np.ndarray) -> np.ndarray:
    """(96,256,64,64) -> (128, 32, 8, 50) bf16: partition=c%128,
    free=(pair, chunk k=(cb,dy,dx), view, region rh*5+rw)."""
    win = np.array([[c - 1, c] for c in _CENTRES])  # (5, 2): rows/cols of window
    v = np.stack([proj[32:64], proj[64:96]], axis=1)  # (32, 2, 256, 64, 64)
    g = v[:, :, :, win[:, :, None, None], win[None, None, :, :]]  # (32,2,256,5,2,5,2)
    g = g.reshape(32, 2, 2, 128, 5, 2, 5, 2)  # b, view, cb, c', rh, dy, rw, dx
    arr = np.transpose(g, (3, 0, 2, 5, 7, 1, 4, 6))  # c', b, cb, dy, dx, view, rh, rw
    return np.ascontiguousarray(arr).reshape(128, 32, 8, 50).astype(ml_dtypes.bfloat16)


def kernel(proj: np.ndarray) -> np.ndarray:
    from concourse.bass_utils import run_bass_kernel_spmd

    nc = get_nc()
    arr = pack_inputs(np.asarray(proj))
    cf = np.eye(50, dtype=np.float32)
    in_maps = [
        {
            "u": np.ascontiguousarray(arr[:, c * NB : (c + 1) * NB]).reshape(128, FREE),
            "cf": cf,
        }
        for c in range(NCORES)
    ]
    results = run_bass_kernel_spmd(nc, in_maps, list(range(NCORES))).results
    total = 0.0
    for r in results:
        out = np.asarray(r["out"], dtype=np.float64)  # (50, 8)
        lnes = out[:, 0:4]          # lse - 10 per (row, pair)
        posf = out[0:25, 4:8]       # positive logits - 10 per (region, pair)
        total += lnes.sum() - 2.0 * posf.sum()
    return np.float32(total / (2.0 * R * NB * NCORES))
